# revision 11
# baseline (speedup 1.0000x reference)
"""MiniTransformerLayer on 8 Trainium2 NeuronCores.

Sharding (single kernel launch, 2 collectives, no all-reduce):
  - tokens t = b*S + s flattened to [4096]; core c owns tokens [512c, 512(c+1))
    and heads {2c, 2c+1} (for both batches).
  - LN1 computed on own token shard (activations kept transposed [hidden, token]),
    AllGather -> full h^T on every core.
  - qkv column-sharded by head. q,k produced feature-major [d, t] (with an
    even/odd d-permutation so RoPE needs no partition swaps), v token-major [t, d].
  - attention per (batch, head): scores computed transposed (s^T[k,q] = k^T.T @ q^T),
    exp on ScalarE (constant -3 bias instead of row-max; cancels in normalization),
    denominator = ones-vector matmul over a DVE-folded chunk accumulator,
    attn@V contracts k directly with p^T as the moving operand -> out [d, q].
  - AllToAll converts head-sharded attn output to token-sharded full-feature.
  - out_proj / MLP computed data-parallel on own 512 tokens with replicated
    (streamed) weights. Residual path in fp32; matmul operands fp16.
"""

import sys

sys.path.insert(0, "/opt/trn_rl_repo")

import numpy as np

import concourse.bass as bass
import concourse.bacc as bacc
import concourse.tile as tile
import concourse.mybir as mybir
from concourse import bass_utils

F16 = mybir.dt.float16
F32 = mybir.dt.float32
AF = mybir.ActivationFunctionType

NCORES = 8
B, S, HID, HEADS, D, FFN = 2, 2048, 2048, 16, 128, 4096
TOK = B * S            # 4096 flat tokens
TPC = TOK // NCORES    # 512 tokens per core
HC = HID // 128        # 16 hidden chunks
FFC = FFN // 128       # 32 ffn chunks
NH = HEADS // NCORES   # 2 heads per core
SCALE = 1.0 / float(np.sqrt(D))
EXP_BIAS = -3.0
EPS = 1e-5

_CACHE = {}


def _emit(nc, single_core=False):
    xT = nc.dram_tensor("xT", [HID, TPC], F32, kind="ExternalInput")
    wq = nc.dram_tensor("wq", [128, HC * NH * 128], F16, kind="ExternalInput")
    wk = nc.dram_tensor("wk", [128, HC * NH * 128], F16, kind="ExternalInput")
    wv = nc.dram_tensor("wv", [128, HC * NH * 128], F16, kind="ExternalInput")
    wo = nc.dram_tensor("wo", [HC * 128, HC * 128], F16, kind="ExternalInput")
    wf1 = nc.dram_tensor("wf1", [FFC * 128, HC * 128], F16, kind="ExternalInput")
    wf2 = nc.dram_tensor("wf2", [HC * 128, FFC * 128], F16, kind="ExternalInput")
    g1 = nc.dram_tensor("g1", [128, HC], F32, kind="ExternalInput")
    b1 = nc.dram_tensor("b1", [128, HC], F32, kind="ExternalInput")
    g2 = nc.dram_tensor("g2", [128, HC], F32, kind="ExternalInput")
    b2 = nc.dram_tensor("b2", [128, HC], F32, kind="ExternalInput")
    ropeC = nc.dram_tensor("ropeC", [128, TOK], F16, kind="ExternalInput")
    ropeS = nc.dram_tensor("ropeS", [128, TOK], F16, kind="ExternalInput")
    outT = nc.dram_tensor("outT", [HID, TPC], F32, kind="ExternalOutput")

    rg = [list(range(NCORES))]
    MULT, ADD = mybir.AluOpType.mult, mybir.AluOpType.add

    with tile.TileContext(nc) as tc:
        with (
            tc.tile_pool(name="const", bufs=1) as const,
            tc.tile_pool(name="dram", bufs=1, space="DRAM") as dram,
        ):
            ones_col = const.tile([128, 1], F32, tag="onc")
            nc.vector.memset(ones_col[:], 1.0)
            ones_col16 = const.tile([128, 1], F16, tag="onc16")
            nc.vector.memset(ones_col16[:], 1.0)
            ones_row = const.tile([1, 128], F32, tag="onr")
            nc.vector.memset(ones_row[:], 1.0)
            eps_b = const.tile([1, 1], F32, tag="epsb")
            nc.vector.memset(eps_b[:], EPS)
            zero1_b = const.tile([1, 1], F32, tag="z1b")
            nc.vector.memset(zero1_b[:], 0.0)
            zero_b = const.tile([128, 1], F32, tag="zb")
            nc.vector.memset(zero_b[:], 0.0)
            expb_b = const.tile([128, 1], F32, tag="expb")
            nc.vector.memset(expb_b[:], EXP_BIAS)
            g1_sb = const.tile([128, HC], F32, tag="g1")
            b1_sb = const.tile([128, HC], F32, tag="b1")
            g2_sb = const.tile([128, HC], F32, tag="g2")
            b2_sb = const.tile([128, HC], F32, tag="b2")
            nc.sync.dma_start(g1_sb[:], g1[:])
            nc.sync.dma_start(b1_sb[:], b1[:])
            nc.sync.dma_start(g2_sb[:], g2[:])
            nc.sync.dma_start(b2_sb[:], b2[:])

            ag_in_a = dram.tile([HID // 2, TPC], F16)
            ag_in_b = dram.tile([HID // 2, TPC], F16)
            ag_out_a = dram.tile([NCORES * HID // 2, TPC], F16)
            ag_out_b = dram.tile([NCORES * HID // 2, TPC], F16)
            a2a_in = dram.tile([NCORES * NH * 128, TPC], F16)
            a2a_out = dram.tile([NCORES * NH * 128, TPC], F16)

            def layernorm(get_src, put_dst, gg, bb, lnp, psst, psbc):
                # h = (x - mu) * rstd * g + b, contraction over partitions via
                # ones-matmuls; per-token coeffs broadcast via K=1 matmuls.
                ps_sx = psst.tile([1, TPC], F32, tag="st")
                ps_sq = psst.tile([1, TPC], F32, tag="st")
                for j in range(HC):
                    s = get_src(j)
                    sqt = lnp.tile([128, TPC], F32, tag="sqt")
                    nc.vector.tensor_mul(sqt[:], s, s)
                    nc.tensor.matmul(ps_sx[:], ones_col[:], s,
                                     start=(j == 0), stop=(j == HC - 1))
                    nc.tensor.matmul(ps_sq[:], ones_col[:], sqt[:],
                                     start=(j == 0), stop=(j == HC - 1))
                mu = lnp.tile([1, TPC], F32, tag="mu")
                m2 = lnp.tile([1, TPC], F32, tag="m2")
                var = lnp.tile([1, TPC], F32, tag="var")
                lnv = lnp.tile([1, TPC], F32, tag="lnv")
                rstd = lnp.tile([1, TPC], F32, tag="rstd")
                mrs = lnp.tile([1, TPC], F32, tag="mrs")
                nc.vector.tensor_scalar_mul(mu[:], ps_sx[:], 1.0 / HID)
                nc.vector.tensor_scalar_mul(m2[:], ps_sq[:], 1.0 / HID)
                nc.vector.tensor_mul(var[:], mu[:], mu[:])
                nc.vector.tensor_sub(var[:], m2[:], var[:])
                nc.scalar.activation(lnv[:], var[:], AF.Ln, bias=eps_b[:])
                nc.scalar.activation(rstd[:], lnv[:], AF.Exp, bias=zero1_b[:],
                                     scale=-0.5)
                nc.vector.tensor_mul(mrs[:], mu[:], rstd[:])
                nc.vector.tensor_scalar_mul(mrs[:], mrs[:], -1.0)
                ps_c1 = psbc.tile([128, TPC], F32, tag="bc")
                ps_c0 = psbc.tile([128, TPC], F32, tag="bc")
                nc.tensor.matmul(ps_c1[:], ones_row[:], rstd[:], start=True, stop=True)
                nc.tensor.matmul(ps_c0[:], ones_row[:], mrs[:], start=True, stop=True)
                for j in range(HC):
                    s = get_src(j)
                    t1 = lnp.tile([128, TPC], F32, tag="t1")
                    t2 = lnp.tile([128, TPC], F32, tag="t2")
                    nc.vector.tensor_mul(t1[:], s, ps_c1[:])
                    nc.vector.tensor_add(t2[:], t1[:], ps_c0[:])
                    put_dst(j, t2, gg[:, j:j + 1], bb[:, j:j + 1])

            # ---------------- Stage A: LN1 (x streamed) + AllGather ----------
            with (
                tc.tile_pool(name="lnA", bufs=3) as lnA,
                tc.tile_pool(name="psstA", bufs=2, space="PSUM") as psstA,
                tc.tile_pool(name="psbcA", bufs=2, space="PSUM") as psbcA,
            ):
                def get_x(j):
                    t = lnA.tile([128, TPC], F32, tag="xs")
                    nc.sync.dma_start(t[:], xT[j * 128:(j + 1) * 128, :])
                    return t[:]

                def put_h1(j, t2, gj, bj):
                    hc_t = lnA.tile([128, TPC], F16, tag="hc")
                    nc.gpsimd.tensor_scalar(hc_t[:], t2[:], gj, bj, MULT, ADD)
                    tgt = ag_in_a if j < 8 else ag_in_b
                    jj = j % 8
                    nc.sync.dma_start(tgt[jj * 128:(jj + 1) * 128, :], hc_t[:])

                layernorm(get_x, put_h1, g1_sb, b1_sb, lnA, psstA, psbcA)

            H2 = HID // 2
            if single_core:
                # timing stand-in for AllGather: replicate shard 8x via DMA
                for r in range(NCORES):
                    nc.sync.dma_start(ag_out_a[r * H2:(r + 1) * H2, :], ag_in_a[:, :])
                    nc.sync.dma_start(ag_out_b[r * H2:(r + 1) * H2, :], ag_in_b[:, :])
            else:
                nc.gpsimd.collective_compute(
                    "AllGather", mybir.AluOpType.bypass, replica_groups=rg,
                    ins=[ag_in_a.opt()], outs=[ag_out_a.opt()],
                )
                nc.gpsimd.collective_compute(
                    "AllGather", mybir.AluOpType.bypass, replica_groups=rg,
                    ins=[ag_in_b.opt()], outs=[ag_out_b.opt()],
                )

            with tc.tile_pool(name="qkv", bufs=1) as qkv:
                qr_sb = qkv.tile([128, NH * TOK], F16, tag="qr")
                kr_sb = qkv.tile([128, NH * TOK], F16, tag="kr")
                v_sb = qkv.tile([128, (TOK // 128) * NH * 128], F16, tag="v")
                rC = qkv.tile([128, TOK], F16, tag="rC")
                rS = qkv.tile([128, TOK], F16, tag="rS")
                nc.sync.dma_start(rC[:], ropeC[:])
                nc.sync.dma_start(rS[:], ropeS[:])
                wq_sb = qkv.tile([128, HC * NH * 128], F16, tag="wq")
                wk_sb = qkv.tile([128, HC * NH * 128], F16, tag="wk")
                wv_sb = qkv.tile([128, HC * NH * 128], F16, tag="wv")
                nc.sync.dma_start(wq_sb[:], wq[:])
                nc.sync.dma_start(wk_sb[:], wk[:])
                nc.sync.dma_start(wv_sb[:], wv[:])

                # ---------------- Stage B: qkv projections + RoPE ------------
                with (
                    tc.tile_pool(name="htc", bufs=26) as htc,
                    tc.tile_pool(name="qkpre", bufs=6) as qkpre,
                    tc.tile_pool(name="ropet", bufs=8) as ropet,
                    tc.tile_pool(name="psqk", bufs=4, space="PSUM") as psqk,
                    tc.tile_pool(name="psv", bufs=4, space="PSUM") as psv,
                ):
                    for tb in range(NCORES):
                        hts = []
                        for j in range(HC):
                            t = htc.tile([128, TPC], F16, tag="ht")
                            buf = ag_out_a if j < 8 else ag_out_b
                            jj = j % 8
                            nc.sync.dma_start(
                                t[:],
                                buf[tb * (HID // 2) + jj * 128:
                                    tb * (HID // 2) + (jj + 1) * 128, :],
                            )
                            hts.append(t)
                        for (w_sb, r_sb) in ((wq_sb, qr_sb), (wk_sb, kr_sb)):
                            for m in range(NH):
                                ps = psqk.tile([128, TPC], F32, tag="qk")
                                for j in range(HC):
                                    nc.tensor.matmul(
                                        ps[:],
                                        w_sb[:, j * (NH * 128) + m * 128:
                                             j * (NH * 128) + (m + 1) * 128],
                                        hts[j][:],
                                        start=(j == 0), stop=(j == HC - 1),
                                    )
                                pre = qkpre.tile([128, TPC], F16, tag="pre")
                                nc.scalar.activation(pre[:], ps[:], AF.Copy)
                                # RoPE: rows [0:64] even dims, [64:128] odd dims
                                col = m * TOK + tb * TPC
                                cs = slice(tb * TPC, (tb + 1) * TPC)
                                qe = pre[0:64, :]
                                qo = pre[64:128, :]
                                t1 = ropet.tile([64, TPC], F16, tag="t1")
                                t2 = ropet.tile([64, TPC], F16, tag="t2")
                                t3 = ropet.tile([64, TPC], F16, tag="t3")
                                t4 = ropet.tile([64, TPC], F16, tag="t4")
                                nc.vector.tensor_mul(t1[:], qe, rC[0:64, cs])
                                nc.vector.tensor_mul(t2[:], qo, rS[64:128, cs])
                                nc.vector.tensor_sub(
                                    r_sb[0:64, col:col + TPC], t1[:], t2[:])
                                nc.vector.tensor_mul(t3[:], qe, rS[0:64, cs])
                                nc.vector.tensor_mul(t4[:], qo, rC[64:128, cs])
                                nc.vector.tensor_add(
                                    r_sb[64:128, col:col + TPC], t3[:], t4[:])
                        for mt in range(4):
                            ps = psv.tile([128, NH * 128], F32, tag="v")
                            for j in range(HC):
                                nc.tensor.matmul(
                                    ps[:],
                                    hts[j][:, mt * 128:(mt + 1) * 128],
                                    wv_sb[:, j * (NH * 128):(j + 1) * (NH * 128)],
                                    start=(j == 0), stop=(j == HC - 1),
                                )
                            ti = tb * 4 + mt
                            nc.scalar.activation(
                                v_sb[:, ti * (NH * 128):(ti + 1) * (NH * 128)],
                                ps[:], AF.Copy)

                # ---------------- Stage C: attention -------------------------
                SB = S // TPC   # 4 query blocks per batch
                KCN = S // 128  # 16 key chunks per batch
                with (
                    tc.tile_pool(name="cp", bufs=4) as cp,
                    tc.tile_pool(name="pss", bufs=2, space="PSUM") as pss_p,
                    tc.tile_pool(name="pso", bufs=3, space="PSUM") as pso_p,
                    tc.tile_pool(name="psdn", bufs=1, space="PSUM") as psdn_p,
                ):
                    for b in range(B):
                        for m in range(NH):
                            qcol = m * TOK + b * S
                            for qb in range(SB):
                                pso = pso_p.tile([128, TPC], F32, tag="o")
                                den = cp.tile([128, TPC], F16, tag="den")
                                for kc in range(KCN):
                                    pss = pss_p.tile([128, TPC], F32, tag="s")
                                    nc.tensor.matmul(
                                        pss[:],
                                        kr_sb[:, qcol + kc * 128: qcol + (kc + 1) * 128],
                                        qr_sb[:, qcol + qb * TPC: qcol + (qb + 1) * TPC],
                                        start=True, stop=True,
                                    )
                                    pt = cp.tile([128, TPC], F16, tag="pt")
                                    nc.scalar.activation(
                                        pt[:], pss[:], AF.Exp, scale=SCALE,
                                        bias=expb_b[:])
                                    if kc == 0:
                                        nc.vector.tensor_copy(den[:], pt[:])
                                    else:
                                        nc.vector.tensor_add(den[:], den[:], pt[:])
                                    ti = b * (S // 128) + kc
                                    nc.tensor.matmul(
                                        pso[:],
                                        v_sb[:, ti * (NH * 128) + m * 128:
                                             ti * (NH * 128) + (m + 1) * 128],
                                        pt[:],
                                        start=(kc == 0), stop=(kc == KCN - 1),
                                    )
                                psden = psdn_p.tile([1, TPC], F32, tag="dn")
                                nc.tensor.matmul(psden[:], ones_col16[:], den[:],
                                                 start=True, stop=True)
                                rec = cp.tile([1, TPC], F32, tag="rec")
                                nc.vector.reciprocal(rec[:], psden[:])
                                rb = cp.tile([128, TPC], F32, tag="rbs")
                                nc.gpsimd.partition_broadcast(rb[:], rec[:])
                                at = cp.tile([128, TPC], F16, tag="at")
                                nc.vector.tensor_mul(at[:], pso[:], rb[:])
                                row = (b * SB + qb) * (NH * 128) + m * 128
                                nc.sync.dma_start(a2a_in[row:row + 128, :], at[:])

            if single_core:
                nc.sync.dma_start(a2a_out[:, :], a2a_in[:, :])
            else:
                nc.gpsimd.collective_compute(
                    "AllToAll", mybir.AluOpType.bypass, replica_groups=rg,
                    ins=[a2a_in.opt()], outs=[a2a_out.opt()],
                )

            with tc.tile_pool(name="late", bufs=1) as late:
                x2_sb = late.tile([128, HC * TPC], F32, tag="x2")
                h2_sb = late.tile([128, HC * TPC], F16, tag="h2")
                ff_sb = late.tile([128, FFC * TPC], F16, tag="ff")

                # ------------- Stage D: out_proj + residual + LN2 ------------
                with (
                    tc.tile_pool(name="atp", bufs=HC + 2) as atp,
                    tc.tile_pool(name="wop", bufs=3) as wop,
                    tc.tile_pool(name="lnD", bufs=3) as lnD,
                    tc.tile_pool(name="pso2", bufs=4, space="PSUM") as pso2_p,
                    tc.tile_pool(name="psstD", bufs=2, space="PSUM") as psstD,
                    tc.tile_pool(name="psbcD", bufs=2, space="PSUM") as psbcD,
                ):
                    ats = []
                    for j in range(HC):
                        t = atp.tile([128, TPC], F16, tag="at")
                        nc.sync.dma_start(t[:], a2a_out[j * 128:(j + 1) * 128, :])
                        ats.append(t)
                    for mo in range(HC):
                        ws = wop.tile([128, HC * 128], F16, tag="wo")
                        nc.sync.dma_start(ws[:], wo[mo * 128:(mo + 1) * 128, :])
                        ps = pso2_p.tile([128, TPC], F32, tag="o2")
                        for j in range(HC):
                            nc.tensor.matmul(
                                ps[:], ws[:, j * 128:(j + 1) * 128], ats[j][:],
                                start=(j == 0), stop=(j == HC - 1),
                            )
                        xt = lnD.tile([128, TPC], F32, tag="xres")
                        nc.sync.dma_start(xt[:], xT[mo * 128:(mo + 1) * 128, :])
                        nc.vector.tensor_add(
                            x2_sb[:, mo * TPC:(mo + 1) * TPC], ps[:], xt[:])

                    def get_x2(j):
                        return x2_sb[:, j * TPC:(j + 1) * TPC]

                    def put_h2(j, t2, gj, bj):
                        nc.gpsimd.tensor_scalar(
                            h2_sb[:, j * TPC:(j + 1) * TPC], t2[:], gj, bj,
                            MULT, ADD)

                    layernorm(get_x2, put_h2, g2_sb, b2_sb, lnD, psstD, psbcD)

                # ------------- Stage E: MLP ----------------------------------
                with (
                    tc.tile_pool(name="wf1p", bufs=4) as wf1p,
                    tc.tile_pool(name="wf2p", bufs=4) as wf2p,
                    tc.tile_pool(name="outp", bufs=3) as outp,
                    tc.tile_pool(name="psf1", bufs=4, space="PSUM") as psf1_p,
                    tc.tile_pool(name="psf2", bufs=4, space="PSUM") as psf2_p,
                ):
                    for mo in range(FFC):
                        ws = wf1p.tile([128, HC * 128], F16, tag="wf1")
                        nc.sync.dma_start(ws[:], wf1[mo * 128:(mo + 1) * 128, :])
                        ps = psf1_p.tile([128, TPC], F32, tag="f1")
                        for j in range(HC):
                            nc.tensor.matmul(
                                ps[:], ws[:, j * 128:(j + 1) * 128],
                                h2_sb[:, j * TPC:(j + 1) * TPC],
                                start=(j == 0), stop=(j == HC - 1),
                            )
                        nc.scalar.activation(
                            ff_sb[:, mo * TPC:(mo + 1) * TPC], ps[:], AF.Gelu,
                            bias=zero_b[:])
                    for mo in range(HC):
                        ws = wf2p.tile([128, FFC * 128], F16, tag="wf2")
                        nc.sync.dma_start(ws[:], wf2[mo * 128:(mo + 1) * 128, :])
                        ps = psf2_p.tile([128, TPC], F32, tag="f2")
                        for j in range(FFC):
                            nc.tensor.matmul(
                                ps[:], ws[:, j * 128:(j + 1) * 128],
                                ff_sb[:, j * TPC:(j + 1) * TPC],
                                start=(j == 0), stop=(j == FFC - 1),
                            )
                        ot = outp.tile([128, TPC], F32, tag="ot")
                        nc.vector.tensor_add(
                            ot[:], ps[:], x2_sb[:, mo * TPC:(mo + 1) * TPC])
                        nc.sync.dma_start(outT[mo * 128:(mo + 1) * 128, :], ot[:])
    return nc


def _build():
    if "nc" in _CACHE:
        return _CACHE["nc"]
    nc = bacc.Bacc(
        "TRN2", target_bir_lowering=False, debug=False,
        enable_asserts=True, num_devices=NCORES,
    )
    _emit(nc)
    nc.compile()
    _CACHE["nc"] = nc
    return nc


def _strips(wT, n_strips):
    # wT [K, n_strips*128] -> [n_strips*128, K] where strip m rows are
    # [128 partitions, K/128 chunks * 128] in SBUF lhsT layout
    K = wT.shape[0]
    kc = K // 128
    out = np.empty((n_strips * 128, K), dtype=np.float16)
    for m_ in range(n_strips):
        s = wT[:, m_ * 128:(m_ + 1) * 128]          # [K, 128]
        s = s.reshape(kc, 128, 128).transpose(1, 0, 2).reshape(128, K)
        out[m_ * 128:(m_ + 1) * 128, :] = s
    return out


def prepare_inputs(x, pe, w_qkv, w_out, w_fc1, w_fc2, g1, b1, g2, b2):
    x = np.asarray(x, np.float32)
    pe = np.asarray(pe, np.float32)
    w_qkv = np.asarray(w_qkv, np.float32)
    w_out = np.asarray(w_out, np.float32)
    w_fc1 = np.asarray(w_fc1, np.float32)
    w_fc2 = np.asarray(w_fc2, np.float32)

    xf = x.reshape(TOK, HID)
    perm = np.r_[np.arange(0, 128, 2), np.arange(1, 128, 2)]

    ropeC = np.tile(pe[:, 0::2].T, (2, B)).astype(np.float16)   # [128, TOK]
    ropeS = np.tile(pe[:, 1::2].T, (2, B)).astype(np.float16)

    gb = [np.asarray(v, np.float32).reshape(HC, 128).T.copy()
          for v in (g1, b1, g2, b2)]

    wo_h = _strips(w_out.T.astype(np.float16), HC)        # w_out.T: [feat, out]
    wf1_h = _strips(w_fc1.T.astype(np.float16), FFC)      # [hid, ffn]
    wf2_h = _strips(w_fc2.T.astype(np.float16), HC)       # [ffn, hid]

    in_maps = []
    for c in range(NCORES):
        heads = [NH * c + i for i in range(NH)]
        # q/k rows with per-head even/odd permutation; v natural
        qrows = np.concatenate([w_qkv[h * D + perm] for h in heads])      # [256, HID]
        krows = np.concatenate([w_qkv[HID + h * D + perm] for h in heads])
        vrows = np.concatenate([w_qkv[2 * HID + h * D: 2 * HID + (h + 1) * D]
                                for h in heads])

        def wlay(rows):
            # rows [NH*128, HID] -> lhsT sbuf layout [128, HC, NH*128]
            t = rows.T.astype(np.float16)                  # [HID, NH*128]
            t = t.reshape(HC, 128, NH * 128).transpose(1, 0, 2)
            return t.reshape(128, HC * NH * 128)

        xTc = np.ascontiguousarray(xf[c * TPC:(c + 1) * TPC].T)  # [HID, TPC]
        in_maps.append({
            "xT": xTc,
            "wq": wlay(qrows), "wk": wlay(krows), "wv": wlay(vrows),
            "wo": wo_h, "wf1": wf1_h, "wf2": wf2_h,
            "g1": gb[0], "b1": gb[1], "g2": gb[2], "b2": gb[3],
            "ropeC": ropeC, "ropeS": ropeS,
        })
    return in_maps


def run(in_maps, **kwargs):
    nc = _build()
    return bass_utils.run_bass_kernel_spmd(
        nc, in_maps, core_ids=list(range(NCORES)), **kwargs
    )


def kernel(x, pe, w_qkv, w_out, w_fc1, w_fc2, g1, b1, g2, b2):
    in_maps = prepare_inputs(x, pe, w_qkv, w_out, w_fc1, w_fc2, g1, b1, g2, b2)
    res = run(in_maps)
    fullT = np.concatenate([res.results[c]["outT"] for c in range(NCORES)], axis=1)
    return np.ascontiguousarray(fullT.T).reshape(B, S, HID).astype(np.float32)


# revision 12
# speedup vs baseline: 1.0069x; 1.0069x over previous
"""MiniTransformerLayer on 8 Trainium2 NeuronCores.

Sharding (single kernel launch, 2 collectives, no all-reduce):
  - tokens t = b*S + s flattened to [4096]; core c owns tokens [512c, 512(c+1))
    and heads {2c, 2c+1} (for both batches).
  - LN1 computed on own token shard (activations kept transposed [hidden, token]),
    AllGather -> full h^T on every core.
  - qkv column-sharded by head. q,k produced feature-major [d, t] (with an
    even/odd d-permutation so RoPE needs no partition swaps), v token-major [t, d].
  - attention per (batch, head): scores computed transposed (s^T[k,q] = k^T.T @ q^T),
    exp on ScalarE (constant -3 bias instead of row-max; cancels in normalization),
    denominator = ones-vector matmul over a DVE-folded chunk accumulator,
    attn@V contracts k directly with p^T as the moving operand -> out [d, q].
  - AllToAll converts head-sharded attn output to token-sharded full-feature.
  - out_proj / MLP computed data-parallel on own 512 tokens with replicated
    (streamed) weights. Residual path in fp32; matmul operands fp16.
"""

import sys

sys.path.insert(0, "/opt/trn_rl_repo")

import numpy as np

import concourse.bass as bass
import concourse.bacc as bacc
import concourse.tile as tile
import concourse.mybir as mybir
from concourse import bass_utils

F16 = mybir.dt.float16
F32 = mybir.dt.float32
AF = mybir.ActivationFunctionType

NCORES = 8
B, S, HID, HEADS, D, FFN = 2, 2048, 2048, 16, 128, 4096
TOK = B * S            # 4096 flat tokens
TPC = TOK // NCORES    # 512 tokens per core
HC = HID // 128        # 16 hidden chunks
FFC = FFN // 128       # 32 ffn chunks
NH = HEADS // NCORES   # 2 heads per core
SCALE = 1.0 / float(np.sqrt(D))
EXP_BIAS = -3.0
EPS = 1e-5

_CACHE = {}


def _emit(nc, single_core=False):
    xT = nc.dram_tensor("xT", [HID, TPC], F32, kind="ExternalInput")
    wq = nc.dram_tensor("wq", [128, HC * NH * 128], F16, kind="ExternalInput")
    wk = nc.dram_tensor("wk", [128, HC * NH * 128], F16, kind="ExternalInput")
    wv = nc.dram_tensor("wv", [128, HC * NH * 128], F16, kind="ExternalInput")
    wo = nc.dram_tensor("wo", [HC * 128, HC * 128], F16, kind="ExternalInput")
    wf1 = nc.dram_tensor("wf1", [FFC * 128, HC * 128], F16, kind="ExternalInput")
    wf2 = nc.dram_tensor("wf2", [HC * 128, FFC * 128], F16, kind="ExternalInput")
    g1 = nc.dram_tensor("g1", [128, HC], F32, kind="ExternalInput")
    b1 = nc.dram_tensor("b1", [128, HC], F32, kind="ExternalInput")
    g2 = nc.dram_tensor("g2", [128, HC], F32, kind="ExternalInput")
    b2 = nc.dram_tensor("b2", [128, HC], F32, kind="ExternalInput")
    ropeC = nc.dram_tensor("ropeC", [128, TOK], F16, kind="ExternalInput")
    ropeS = nc.dram_tensor("ropeS", [128, TOK], F16, kind="ExternalInput")
    outT = nc.dram_tensor("outT", [HID, TPC], F32, kind="ExternalOutput")

    rg = [list(range(NCORES))]
    MULT, ADD = mybir.AluOpType.mult, mybir.AluOpType.add

    with tile.TileContext(nc) as tc:
        with (
            tc.tile_pool(name="const", bufs=1) as const,
            tc.tile_pool(name="dram", bufs=1, space="DRAM") as dram,
        ):
            ones_col = const.tile([128, 1], F32, tag="onc")
            nc.vector.memset(ones_col[:], 1.0)
            ones_col16 = const.tile([128, 1], F16, tag="onc16")
            nc.vector.memset(ones_col16[:], 1.0)
            ones_row = const.tile([1, 128], F32, tag="onr")
            nc.vector.memset(ones_row[:], 1.0)
            eps_b = const.tile([1, 1], F32, tag="epsb")
            nc.vector.memset(eps_b[:], EPS)
            zero1_b = const.tile([1, 1], F32, tag="z1b")
            nc.vector.memset(zero1_b[:], 0.0)
            zero_b = const.tile([128, 1], F32, tag="zb")
            nc.vector.memset(zero_b[:], 0.0)
            expb_b = const.tile([128, 1], F32, tag="expb")
            nc.vector.memset(expb_b[:], EXP_BIAS)
            g1_sb = const.tile([128, HC], F32, tag="g1")
            b1_sb = const.tile([128, HC], F32, tag="b1")
            g2_sb = const.tile([128, HC], F32, tag="g2")
            b2_sb = const.tile([128, HC], F32, tag="b2")
            nc.sync.dma_start(g1_sb[:], g1[:])
            nc.sync.dma_start(b1_sb[:], b1[:])
            nc.sync.dma_start(g2_sb[:], g2[:])
            nc.sync.dma_start(b2_sb[:], b2[:])

            ag_in_a = dram.tile([HID // 2, TPC], F16)
            ag_in_b = dram.tile([HID // 2, TPC], F16)
            ag_out_a = dram.tile([NCORES * HID // 2, TPC], F16)
            ag_out_b = dram.tile([NCORES * HID // 2, TPC], F16)
            a2a_in = dram.tile([NCORES * NH * 128, TPC], F16)
            a2a_out = dram.tile([NCORES * NH * 128, TPC], F16)

            def layernorm(get_src, put_dst, gg, bb, lnp, psst, psbc):
                # h = (x - mu) * rstd * g + b, contraction over partitions via
                # ones-matmuls; per-token coeffs broadcast via K=1 matmuls.
                ps_sx = psst.tile([1, TPC], F32, tag="st")
                ps_sq = psst.tile([1, TPC], F32, tag="st")
                for j in range(HC):
                    s = get_src(j)
                    sqt = lnp.tile([128, TPC], F32, tag="sqt")
                    nc.vector.tensor_mul(sqt[:], s, s)
                    nc.tensor.matmul(ps_sx[:], ones_col[:], s,
                                     start=(j == 0), stop=(j == HC - 1))
                    nc.tensor.matmul(ps_sq[:], ones_col[:], sqt[:],
                                     start=(j == 0), stop=(j == HC - 1))
                mu = lnp.tile([1, TPC], F32, tag="mu")
                m2 = lnp.tile([1, TPC], F32, tag="m2")
                var = lnp.tile([1, TPC], F32, tag="var")
                lnv = lnp.tile([1, TPC], F32, tag="lnv")
                rstd = lnp.tile([1, TPC], F32, tag="rstd")
                mrs = lnp.tile([1, TPC], F32, tag="mrs")
                nc.vector.tensor_scalar_mul(mu[:], ps_sx[:], 1.0 / HID)
                nc.vector.tensor_scalar_mul(m2[:], ps_sq[:], 1.0 / HID)
                nc.vector.tensor_mul(var[:], mu[:], mu[:])
                nc.vector.tensor_sub(var[:], m2[:], var[:])
                nc.scalar.activation(lnv[:], var[:], AF.Ln, bias=eps_b[:])
                nc.scalar.activation(rstd[:], lnv[:], AF.Exp, bias=zero1_b[:],
                                     scale=-0.5)
                nc.vector.tensor_mul(mrs[:], mu[:], rstd[:])
                nc.vector.tensor_scalar_mul(mrs[:], mrs[:], -1.0)
                ps_c1 = psbc.tile([128, TPC], F32, tag="bc")
                ps_c0 = psbc.tile([128, TPC], F32, tag="bc")
                nc.tensor.matmul(ps_c1[:], ones_row[:], rstd[:], start=True, stop=True)
                nc.tensor.matmul(ps_c0[:], ones_row[:], mrs[:], start=True, stop=True)
                for j in range(HC):
                    s = get_src(j)
                    t1 = lnp.tile([128, TPC], F32, tag="t1")
                    t2 = lnp.tile([128, TPC], F32, tag="t2")
                    nc.vector.tensor_mul(t1[:], s, ps_c1[:])
                    nc.vector.tensor_add(t2[:], t1[:], ps_c0[:])
                    put_dst(j, t2, gg[:, j:j + 1], bb[:, j:j + 1])

            # ---------------- Stage A: LN1 (x streamed) + AllGather ----------
            with (
                tc.tile_pool(name="lnA", bufs=3) as lnA,
                tc.tile_pool(name="psstA", bufs=2, space="PSUM") as psstA,
                tc.tile_pool(name="psbcA", bufs=2, space="PSUM") as psbcA,
            ):
                def get_x(j):
                    t = lnA.tile([128, TPC], F32, tag="xs")
                    nc.sync.dma_start(t[:], xT[j * 128:(j + 1) * 128, :])
                    return t[:]

                def put_h1(j, t2, gj, bj):
                    hc_t = lnA.tile([128, TPC], F16, tag="hc")
                    nc.gpsimd.tensor_scalar(hc_t[:], t2[:], gj, bj, MULT, ADD)
                    tgt = ag_in_a if j < 8 else ag_in_b
                    jj = j % 8
                    nc.sync.dma_start(tgt[jj * 128:(jj + 1) * 128, :], hc_t[:])

                layernorm(get_x, put_h1, g1_sb, b1_sb, lnA, psstA, psbcA)

            H2 = HID // 2
            if single_core:
                # timing stand-in for AllGather: replicate shard 8x via DMA
                for r in range(NCORES):
                    nc.sync.dma_start(ag_out_a[r * H2:(r + 1) * H2, :], ag_in_a[:, :])
                    nc.sync.dma_start(ag_out_b[r * H2:(r + 1) * H2, :], ag_in_b[:, :])
            else:
                nc.gpsimd.collective_compute(
                    "AllGather", mybir.AluOpType.bypass, replica_groups=rg,
                    ins=[ag_in_a.opt()], outs=[ag_out_a.opt()],
                )
                nc.gpsimd.collective_compute(
                    "AllGather", mybir.AluOpType.bypass, replica_groups=rg,
                    ins=[ag_in_b.opt()], outs=[ag_out_b.opt()],
                )

            with tc.tile_pool(name="qkv", bufs=1) as qkv:
                qr_sb = qkv.tile([128, NH * TOK], F16, tag="qr")
                kr_sb = qkv.tile([128, NH * TOK], F16, tag="kr")
                v_sb = qkv.tile([128, (TOK // 128) * NH * 128], F16, tag="v")
                rC = qkv.tile([128, TOK], F16, tag="rC")
                rS = qkv.tile([128, TOK], F16, tag="rS")
                nc.sync.dma_start(rC[:], ropeC[:])
                nc.sync.dma_start(rS[:], ropeS[:])
                wq_sb = qkv.tile([128, HC * NH * 128], F16, tag="wq")
                wk_sb = qkv.tile([128, HC * NH * 128], F16, tag="wk")
                wv_sb = qkv.tile([128, HC * NH * 128], F16, tag="wv")
                nc.sync.dma_start(wq_sb[:], wq[:])
                nc.sync.dma_start(wk_sb[:], wk[:])
                nc.sync.dma_start(wv_sb[:], wv[:])

                # ---------------- Stage B: qkv projections + RoPE ------------
                with (
                    tc.tile_pool(name="htc", bufs=30) as htc,
                    tc.tile_pool(name="qkpre", bufs=6) as qkpre,
                    tc.tile_pool(name="ropet", bufs=8) as ropet,
                    tc.tile_pool(name="psqk", bufs=4, space="PSUM") as psqk,
                    tc.tile_pool(name="psv", bufs=4, space="PSUM") as psv,
                ):
                    for tb in range(NCORES):
                        hts = []
                        for j in range(HC):
                            t = htc.tile([128, TPC], F16, tag="ht")
                            buf = ag_out_a if j < 8 else ag_out_b
                            jj = j % 8
                            nc.sync.dma_start(
                                t[:],
                                buf[tb * (HID // 2) + jj * 128:
                                    tb * (HID // 2) + (jj + 1) * 128, :],
                            )
                            hts.append(t)
                        for (w_sb, r_sb) in ((wq_sb, qr_sb), (wk_sb, kr_sb)):
                            for m in range(NH):
                                ps = psqk.tile([128, TPC], F32, tag="qk")
                                for j in range(HC):
                                    nc.tensor.matmul(
                                        ps[:],
                                        w_sb[:, j * (NH * 128) + m * 128:
                                             j * (NH * 128) + (m + 1) * 128],
                                        hts[j][:],
                                        start=(j == 0), stop=(j == HC - 1),
                                    )
                                pre = qkpre.tile([128, TPC], F16, tag="pre")
                                nc.scalar.activation(pre[:], ps[:], AF.Copy)
                                # RoPE: rows [0:64] even dims, [64:128] odd dims
                                col = m * TOK + tb * TPC
                                cs = slice(tb * TPC, (tb + 1) * TPC)
                                qe = pre[0:64, :]
                                qo = pre[64:128, :]
                                t1 = ropet.tile([64, TPC], F16, tag="t1")
                                t2 = ropet.tile([64, TPC], F16, tag="t2")
                                t3 = ropet.tile([64, TPC], F16, tag="t3")
                                t4 = ropet.tile([64, TPC], F16, tag="t4")
                                nc.vector.tensor_mul(t1[:], qe, rC[0:64, cs])
                                nc.vector.tensor_mul(t2[:], qo, rS[64:128, cs])
                                nc.vector.tensor_sub(
                                    r_sb[0:64, col:col + TPC], t1[:], t2[:])
                                nc.vector.tensor_mul(t3[:], qe, rS[0:64, cs])
                                nc.vector.tensor_mul(t4[:], qo, rC[64:128, cs])
                                nc.vector.tensor_add(
                                    r_sb[64:128, col:col + TPC], t3[:], t4[:])
                        for mt in range(4):
                            ps = psv.tile([128, NH * 128], F32, tag="v")
                            for j in range(HC):
                                nc.tensor.matmul(
                                    ps[:],
                                    hts[j][:, mt * 128:(mt + 1) * 128],
                                    wv_sb[:, j * (NH * 128):(j + 1) * (NH * 128)],
                                    start=(j == 0), stop=(j == HC - 1),
                                )
                            ti = tb * 4 + mt
                            nc.scalar.activation(
                                v_sb[:, ti * (NH * 128):(ti + 1) * (NH * 128)],
                                ps[:], AF.Copy)

                # ---------------- Stage C: attention -------------------------
                SB = S // TPC   # 4 query blocks per batch
                KCN = S // 128  # 16 key chunks per batch
                with (
                    tc.tile_pool(name="cp", bufs=5) as cp,
                    tc.tile_pool(name="pss", bufs=2, space="PSUM") as pss_p,
                    tc.tile_pool(name="pso", bufs=3, space="PSUM") as pso_p,
                    tc.tile_pool(name="psdn", bufs=1, space="PSUM") as psdn_p,
                ):
                    for b in range(B):
                        for m in range(NH):
                            qcol = m * TOK + b * S
                            for qb in range(SB):
                                pso = pso_p.tile([128, TPC], F32, tag="o")
                                den = cp.tile([128, TPC], F16, tag="den")
                                for kc in range(KCN):
                                    pss = pss_p.tile([128, TPC], F32, tag="s")
                                    nc.tensor.matmul(
                                        pss[:],
                                        kr_sb[:, qcol + kc * 128: qcol + (kc + 1) * 128],
                                        qr_sb[:, qcol + qb * TPC: qcol + (qb + 1) * TPC],
                                        start=True, stop=True,
                                    )
                                    pt = cp.tile([128, TPC], F16, tag="pt")
                                    nc.scalar.activation(
                                        pt[:], pss[:], AF.Exp, scale=SCALE,
                                        bias=expb_b[:])
                                    if kc == 0:
                                        nc.vector.tensor_copy(den[:], pt[:])
                                    else:
                                        nc.vector.tensor_add(den[:], den[:], pt[:])
                                    ti = b * (S // 128) + kc
                                    nc.tensor.matmul(
                                        pso[:],
                                        v_sb[:, ti * (NH * 128) + m * 128:
                                             ti * (NH * 128) + (m + 1) * 128],
                                        pt[:],
                                        start=(kc == 0), stop=(kc == KCN - 1),
                                    )
                                psden = psdn_p.tile([1, TPC], F32, tag="dn")
                                nc.tensor.matmul(psden[:], ones_col16[:], den[:],
                                                 start=True, stop=True)
                                rec = cp.tile([1, TPC], F32, tag="rec")
                                nc.vector.reciprocal(rec[:], psden[:])
                                rb = cp.tile([128, TPC], F32, tag="rbs")
                                nc.gpsimd.partition_broadcast(rb[:], rec[:])
                                at = cp.tile([128, TPC], F16, tag="at")
                                nc.vector.tensor_mul(at[:], pso[:], rb[:])
                                row = (b * SB + qb) * (NH * 128) + m * 128
                                nc.sync.dma_start(a2a_in[row:row + 128, :], at[:])

            if single_core:
                nc.sync.dma_start(a2a_out[:, :], a2a_in[:, :])
            else:
                nc.gpsimd.collective_compute(
                    "AllToAll", mybir.AluOpType.bypass, replica_groups=rg,
                    ins=[a2a_in.opt()], outs=[a2a_out.opt()],
                )

            with tc.tile_pool(name="late", bufs=1) as late:
                x2_sb = late.tile([128, HC * TPC], F32, tag="x2")
                h2_sb = late.tile([128, HC * TPC], F16, tag="h2")
                ff_sb = late.tile([128, FFC * TPC], F16, tag="ff")

                # ------------- Stage D: out_proj + residual + LN2 ------------
                with (
                    tc.tile_pool(name="atp", bufs=HC + 2) as atp,
                    tc.tile_pool(name="wop", bufs=4) as wop,
                    tc.tile_pool(name="lnD", bufs=4) as lnD,
                    tc.tile_pool(name="pso2", bufs=4, space="PSUM") as pso2_p,
                    tc.tile_pool(name="psstD", bufs=2, space="PSUM") as psstD,
                    tc.tile_pool(name="psbcD", bufs=2, space="PSUM") as psbcD,
                ):
                    ats = []
                    for j in range(HC):
                        t = atp.tile([128, TPC], F16, tag="at")
                        nc.sync.dma_start(t[:], a2a_out[j * 128:(j + 1) * 128, :])
                        ats.append(t)
                    for mo in range(HC):
                        ws = wop.tile([128, HC * 128], F16, tag="wo")
                        nc.sync.dma_start(ws[:], wo[mo * 128:(mo + 1) * 128, :])
                        ps = pso2_p.tile([128, TPC], F32, tag="o2")
                        for j in range(HC):
                            nc.tensor.matmul(
                                ps[:], ws[:, j * 128:(j + 1) * 128], ats[j][:],
                                start=(j == 0), stop=(j == HC - 1),
                            )
                        xt = lnD.tile([128, TPC], F32, tag="xres")
                        nc.sync.dma_start(xt[:], xT[mo * 128:(mo + 1) * 128, :])
                        nc.vector.tensor_add(
                            x2_sb[:, mo * TPC:(mo + 1) * TPC], ps[:], xt[:])

                    def get_x2(j):
                        return x2_sb[:, j * TPC:(j + 1) * TPC]

                    def put_h2(j, t2, gj, bj):
                        nc.gpsimd.tensor_scalar(
                            h2_sb[:, j * TPC:(j + 1) * TPC], t2[:], gj, bj,
                            MULT, ADD)

                    layernorm(get_x2, put_h2, g2_sb, b2_sb, lnD, psstD, psbcD)

                # ------------- Stage E: MLP ----------------------------------
                with (
                    tc.tile_pool(name="wf1p", bufs=4) as wf1p,
                    tc.tile_pool(name="wf2p", bufs=5) as wf2p,
                    tc.tile_pool(name="outp", bufs=3) as outp,
                    tc.tile_pool(name="psf1", bufs=4, space="PSUM") as psf1_p,
                    tc.tile_pool(name="psf2", bufs=4, space="PSUM") as psf2_p,
                ):
                    for mo in range(FFC):
                        ws = wf1p.tile([128, HC * 128], F16, tag="wf1")
                        nc.sync.dma_start(ws[:], wf1[mo * 128:(mo + 1) * 128, :])
                        ps = psf1_p.tile([128, TPC], F32, tag="f1")
                        for j in range(HC):
                            nc.tensor.matmul(
                                ps[:], ws[:, j * 128:(j + 1) * 128],
                                h2_sb[:, j * TPC:(j + 1) * TPC],
                                start=(j == 0), stop=(j == HC - 1),
                            )
                        nc.scalar.activation(
                            ff_sb[:, mo * TPC:(mo + 1) * TPC], ps[:], AF.Gelu,
                            bias=zero_b[:])
                    for mo in range(HC):
                        ws = wf2p.tile([128, FFC * 128], F16, tag="wf2")
                        nc.sync.dma_start(ws[:], wf2[mo * 128:(mo + 1) * 128, :])
                        ps = psf2_p.tile([128, TPC], F32, tag="f2")
                        for j in range(FFC):
                            nc.tensor.matmul(
                                ps[:], ws[:, j * 128:(j + 1) * 128],
                                ff_sb[:, j * TPC:(j + 1) * TPC],
                                start=(j == 0), stop=(j == FFC - 1),
                            )
                        ot = outp.tile([128, TPC], F32, tag="ot")
                        nc.vector.tensor_add(
                            ot[:], ps[:], x2_sb[:, mo * TPC:(mo + 1) * TPC])
                        nc.sync.dma_start(outT[mo * 128:(mo + 1) * 128, :], ot[:])
    return nc


def _build():
    if "nc" in _CACHE:
        return _CACHE["nc"]
    nc = bacc.Bacc(
        "TRN2", target_bir_lowering=False, debug=False,
        enable_asserts=True, num_devices=NCORES,
    )
    _emit(nc)
    nc.compile()
    _CACHE["nc"] = nc
    return nc


def _strips(wT, n_strips):
    # wT [K, n_strips*128] -> [n_strips*128, K] where strip m rows are
    # [128 partitions, K/128 chunks * 128] in SBUF lhsT layout
    K = wT.shape[0]
    kc = K // 128
    out = np.empty((n_strips * 128, K), dtype=np.float16)
    for m_ in range(n_strips):
        s = wT[:, m_ * 128:(m_ + 1) * 128]          # [K, 128]
        s = s.reshape(kc, 128, 128).transpose(1, 0, 2).reshape(128, K)
        out[m_ * 128:(m_ + 1) * 128, :] = s
    return out


def prepare_inputs(x, pe, w_qkv, w_out, w_fc1, w_fc2, g1, b1, g2, b2):
    x = np.asarray(x, np.float32)
    pe = np.asarray(pe, np.float32)
    w_qkv = np.asarray(w_qkv, np.float32)
    w_out = np.asarray(w_out, np.float32)
    w_fc1 = np.asarray(w_fc1, np.float32)
    w_fc2 = np.asarray(w_fc2, np.float32)

    xf = x.reshape(TOK, HID)
    perm = np.r_[np.arange(0, 128, 2), np.arange(1, 128, 2)]

    ropeC = np.tile(pe[:, 0::2].T, (2, B)).astype(np.float16)   # [128, TOK]
    ropeS = np.tile(pe[:, 1::2].T, (2, B)).astype(np.float16)

    gb = [np.asarray(v, np.float32).reshape(HC, 128).T.copy()
          for v in (g1, b1, g2, b2)]

    wo_h = _strips(w_out.T.astype(np.float16), HC)        # w_out.T: [feat, out]
    wf1_h = _strips(w_fc1.T.astype(np.float16), FFC)      # [hid, ffn]
    wf2_h = _strips(w_fc2.T.astype(np.float16), HC)       # [ffn, hid]

    in_maps = []
    for c in range(NCORES):
        heads = [NH * c + i for i in range(NH)]
        # q/k rows with per-head even/odd permutation; v natural
        qrows = np.concatenate([w_qkv[h * D + perm] for h in heads])      # [256, HID]
        krows = np.concatenate([w_qkv[HID + h * D + perm] for h in heads])
        vrows = np.concatenate([w_qkv[2 * HID + h * D: 2 * HID + (h + 1) * D]
                                for h in heads])

        def wlay(rows):
            # rows [NH*128, HID] -> lhsT sbuf layout [128, HC, NH*128]
            t = rows.T.astype(np.float16)                  # [HID, NH*128]
            t = t.reshape(HC, 128, NH * 128).transpose(1, 0, 2)
            return t.reshape(128, HC * NH * 128)

        xTc = np.ascontiguousarray(xf[c * TPC:(c + 1) * TPC].T)  # [HID, TPC]
        in_maps.append({
            "xT": xTc,
            "wq": wlay(qrows), "wk": wlay(krows), "wv": wlay(vrows),
            "wo": wo_h, "wf1": wf1_h, "wf2": wf2_h,
            "g1": gb[0], "b1": gb[1], "g2": gb[2], "b2": gb[3],
            "ropeC": ropeC, "ropeS": ropeS,
        })
    return in_maps


def run(in_maps, **kwargs):
    nc = _build()
    return bass_utils.run_bass_kernel_spmd(
        nc, in_maps, core_ids=list(range(NCORES)), **kwargs
    )


def kernel(x, pe, w_qkv, w_out, w_fc1, w_fc2, g1, b1, g2, b2):
    in_maps = prepare_inputs(x, pe, w_qkv, w_out, w_fc1, w_fc2, g1, b1, g2, b2)
    res = run(in_maps)
    fullT = np.concatenate([res.results[c]["outT"] for c in range(NCORES)], axis=1)
    return np.ascontiguousarray(fullT.T).reshape(B, S, HID).astype(np.float32)
